# revision 65
# baseline (speedup 1.0000x reference)
"""BertSelfAttention (B=2, S=2048, HID=1024, NH=16, HD=64, SKV=2048) on 8 TRN2 NeuronCores.

Sharding: tensor-parallel over heads — 2 heads per core. Each core projects its
own 128 output channels of Q/K/V from the full hidden states, runs attention for
its 2 heads against the (sharded) kv cache + fresh K/V, and writes a [B, 128, S]
transposed context slice. The host concatenates the 8 slices along hidden dim.

On-device layout (per core):
  - qT/kT: [128 (2 heads x 64 dims), B*S] with head h on partitions h*64:(h+1)*64.
    Head 0 / head 1 matmuls use PE row-tiles (64,0)/(64,64 base) in parallel.
  - scores computed transposed: scoresT[kv, q] = kT_chunk.T-contract @ qT,
    softmax denominators via an all-ones column appended to V (M=65 ctx matmul).
  - all matmul operands use float32r (full-rate fp32-reduced mode, err ~1e-5).
"""

import sys

sys.path.insert(0, "/opt/trn_rl_repo")

import numpy as np

B, S, HID, NH, HD, SKV = 2, 2048, 1024, 16, 64, 2048
NCORES = 8
P = 128
SC = 512                    # q-chunk width (fp32 moving operand max)
NSC = B * S // SC           # 8 column chunks of hsT
KO = HID // P               # 8 contraction chunks for projections
NJ = (SKV + S) // P         # 32 kv chunks per (b, h); 0..15 cache, 16..31 new
VJ = SKV // P               # 16 chunks per segment
NM = S // SC                # 4 q-chunks per batch
EXP_GROUPS = [3] * 10 + [2]  # kv-chunk grouping for exp ops (3*10+2 == NJ)

_prog_cache = {}


def _build_program():
    import concourse.bacc as bacc
    import concourse.mybir as mybir
    import concourse.tile as tile
    from concourse.masks import make_identity

    f32 = mybir.dt.float32
    f32r = mybir.dt.float32r
    Exp = mybir.ActivationFunctionType.Exp

    nc = bacc.Bacc("TRN2", target_bir_lowering=False, debug=False, num_devices=NCORES)

    hsT = nc.dram_tensor("hsT", [HID, B * S], f32r, kind="ExternalInput").ap()
    wq = nc.dram_tensor("wq", [HID, P], f32r, kind="ExternalInput").ap()
    wk = nc.dram_tensor("wk", [HID, P], f32r, kind="ExternalInput").ap()
    wv = nc.dram_tensor("wv", [HID, P], f32r, kind="ExternalInput").ap()
    bq = nc.dram_tensor("bq", [P, 1], f32, kind="ExternalInput").ap()
    bk = nc.dram_tensor("bk", [P, 1], f32, kind="ExternalInput").ap()
    bv = nc.dram_tensor("bv", [P, 1], f32, kind="ExternalInput").ap()
    onesp = nc.dram_tensor("onesp", [P, 1], f32r, kind="ExternalInput").ap()
    ktc = nc.dram_tensor("ktc", [P, B, SKV], f32r, kind="ExternalInput").ap()
    vc = nc.dram_tensor("vc", [B, SKV, 130], f32r, kind="ExternalInput").ap()
    out = nc.dram_tensor("out", [B, P, S], f32, kind="ExternalOutput").ap()

    with tile.TileContext(nc) as tc:
        with tc.tile_pool(name="persist", bufs=1) as persist:
            # only q weights/bias queue before the first hsT chunks; k/v
            # weights follow the k-cache DMA (not needed until after the
            # first cache-scores are in flight)
            wq_sb = persist.tile([P, KO, P], f32r, tag="wq")
            wk_sb = persist.tile([P, KO, P], f32r, tag="wk")
            wv_sb = persist.tile([P, KO, P], f32r, tag="wv")
            bq_sb = persist.tile([P, 1], f32, tag="bq")
            bk_sb = persist.tile([P, 1], f32, tag="bk")
            bv_sb = persist.tile([P, 1], f32, tag="bv")
            nc.sync.dma_start(wq_sb[:], wq.rearrange("(ko p) m -> p ko m", p=P))
            nc.sync.dma_start(bq_sb[:], bq)
            ktc_sb = persist.tile([P, B, SKV], f32r, tag="ktc")
            # v layout: [p, b, seg, jo, 130]; cols 0:64 head0, 64 ones, 65:129 head1, 129 ones
            v_sb = persist.tile([P, B, 2, VJ, 130], f32r, tag="v")
            ones_sb = persist.tile([P, 1], f32r, tag="ones")

            qT_sb = persist.tile([P, NSC, SC], f32r, tag="qT")
            kTn_sb = persist.tile([P, NSC, SC], f32r, tag="kTn")
            identity = persist.tile([P, P], f32, tag="ident")
            make_identity(nc, identity[:])
            # dummy 1-element exp: hoists the ACT table load to t~0, hiding
            # its ~1.3us under the initial input DMAs
            warm = persist.tile([1, 1], f32, tag="warm")
            nc.scalar.activation(warm[:], identity[0:1, 0:1], Exp, scale=1.0)

            # Phase 1 (projections) and phase 2 (attention) are interleaved in
            # EMISSION order — Tile dependencies follow program order, so every
            # consumer must be emitted after its producer. Batch-0 attention
            # starts on the kv cache as soon as ktc + the first q chunk exist,
            # which gets the exp stream on ACT (the saturated engine) going
            # ~50us earlier than sequential phases. PSUM is fully booked by
            # attention (2 heads x 3-bank scores + 2 ctx accumulators = 8
            # banks), so projection matmuls borrow the scores-pool slots.
            hsT_r = hsT.rearrange("(ko p) n -> p ko n", p=P)
            qT_w = qT_sb[:].rearrange("p a b -> p (a b)")
            kTn_w = kTn_sb[:].rearrange("p a b -> p (a b)")
            qT_f = qT_w
            kTn_f = kTn_w
            # first chunks narrowed so the first matmuls start sooner;
            # chunks 0-4 cover batch 0 (cols 0:2048), chunks 5-8 batch 1
            chunks = [(0, 256), (256, 256)] + [(i * SC, SC) for i in range(1, NSC)]
            with (
                tc.tile_pool(name="hst", bufs=2) as hpool,
                tc.tile_pool(name="vt", bufs=2) as vtp,
                tc.tile_pool(name="scps", bufs=1, space="PSUM") as scps,
                tc.tile_pool(name="ctxps", bufs=1, space="PSUM") as ctxps,
                tc.tile_pool(name="probs", bufs=4) as probp,
                tc.tile_pool(name="norm", bufs=2) as normp,
            ):

                def sc_psum(slot):
                    t = scps.tile([P, 3, SC], f32, tag=f"sc{slot}", name="p1ps")
                    return t[:, 0]

                p1_hst = {}

                def _p1_proj(ci, slot, w_sb, b_sb, dest):
                    off, cw = chunks[ci]
                    ps = sc_psum(slot)[:, :cw]
                    for ko in range(KO):
                        nc.tensor.matmul(
                            ps, w_sb[:, ko], p1_hst[ci][:, ko, :cw],
                            start=(ko == 0), stop=(ko == KO - 1),
                        )
                    nc.vector.tensor_add(
                        dest[:, off:off + cw], ps, b_sb[:].to_broadcast((P, cw))
                    )

                def emit_p1_q(ci):
                    off, cw = chunks[ci]
                    hst = hpool.tile([P, KO, SC], f32r, tag="hst", name="hst")
                    p1_hst[ci] = hst
                    nc.sync.dma_start(hst[:, :, :cw], hsT_r[:, :, off:off + cw])
                    _p1_proj(ci, 0, wq_sb, bq_sb, qT_w)

                def emit_p1_qk(ci):
                    emit_p1_q(ci)
                    _p1_proj(ci, 1, wk_sb, bk_sb, kTn_w)

                def emit_p1_v(ci):
                    # V: project transposed, then PE-transpose into row layout
                    off, cw = chunks[ci]
                    ps = sc_psum(0)[:, :cw]
                    hst = p1_hst.pop(ci)
                    for ko in range(KO):
                        nc.tensor.matmul(
                            ps, wv_sb[:, ko], hst[:, ko, :cw],
                            start=(ko == 0), stop=(ko == KO - 1),
                        )
                    vt = vtp.tile([P, SC], f32, tag="vt", name="vt")
                    nc.vector.tensor_add(
                        vt[:, :cw], ps, bv_sb[:].to_broadcast((P, cw))
                    )
                    for t in range(cw // P):
                        tp = sc_psum(1)[:, :P]
                        nc.tensor.transpose(tp, vt[:, t * P:(t + 1) * P], identity[:])
                        base = off + t * P
                        b_i, jo = base // S, (base % S) // P
                        nc.vector.tensor_copy(out=v_sb[:, b_i, 1, jo, 0:64], in_=tp[:, 0:64])
                        nc.vector.tensor_copy(out=v_sb[:, b_i, 1, jo, 65:129], in_=tp[:, 64:128])

                def emit_p1_chunk(ci):
                    emit_p1_qk(ci)
                    emit_p1_v(ci)

                p2_state = {}

                def p2_start(b, m):
                    p2_state[(b, m)] = {
                        "ctx": [
                            ctxps.tile([P, SC], f32, tag=f"ctx{h}", name=f"ctx{h}")
                            for h in range(2)
                        ],
                        "pending": [],
                        "j": 0,
                        "gi": 0,
                    }

                def p2_groups(b, m, ngroups):
                    st = p2_state[(b, m)]
                    q0 = b * S + m * SC
                    ctx = st["ctx"]

                    def emit_ctx(h, j0, g, pr):
                        for jj in range(g):
                            jg = j0 + jj
                            seg, jo = (0, jg) if jg < VJ else (1, jg - VJ)
                            nc.tensor.matmul(
                                ctx[h][0:65, :],
                                v_sb[:, b, seg, jo, h * 65:(h + 1) * 65],
                                pr[:, jj],
                                start=(jg == 0), stop=(jg == NJ - 1),
                            )

                    for g in EXP_GROUPS[st["gi"]:st["gi"] + ngroups]:
                        j = st["j"]
                        nxt = []
                        sct = [
                            scps.tile([P, 3, SC], f32, tag=f"sc{h}", name=f"sc{h}")
                            for h in range(2)
                        ]
                        # head-BLOCKED order: h0's scores only gate on h0's
                        # previous exp, so exp(g,h0) is ready the moment ACT
                        # finishes exp(g-1,h1) — interleaving the heads would
                        # park h0's last matmul behind h1's slot wait in the
                        # in-order PE stream, bubbling ACT every group. The
                        # two heads still land on PE row-tiles (0,*)/(64,*).
                        for h in range(2):
                            hs0, hs1 = h * 64, (h + 1) * 64
                            for jj in range(g):
                                jg = j + jj
                                if jg < VJ:
                                    lhsT = ktc_sb[hs0:hs1, b, jg * P:(jg + 1) * P]
                                else:
                                    col = b * S + (jg - VJ) * P
                                    lhsT = kTn_f[hs0:hs1, col:col + P]
                                nc.tensor.matmul(
                                    sct[h][:, jj], lhsT, qT_f[hs0:hs1, q0:q0 + SC],
                                    start=True, stop=True,
                                )
                        for h in range(2):
                            pr = probp.tile([P, 3, SC], f32r, tag=f"pr{h}")
                            nc.scalar.activation(
                                pr[:, :g], sct[h][:, :g], Exp, scale=0.125
                            )
                            nxt.append((h, j, g, pr))
                        # ctx trails scores/exp by two groups: PE stays ahead
                        # and score->ctx mode transitions come in longer runs
                        st["pending"].append(nxt)
                        if len(st["pending"]) > 2:
                            for args in st["pending"].pop(0):
                                emit_ctx(*args)
                        st["j"] = j + g
                        st["gi"] += 1

                    if st["gi"] == len(EXP_GROUPS):
                        for batch in st["pending"]:
                            for args in batch:
                                emit_ctx(*args)
                        st["pending"] = []
                        for h in range(2):
                            # one quick copy releases the ctx PSUM bank early
                            tmp = normp.tile([65, SC], f32, tag="tmp")
                            nc.vector.tensor_copy(out=tmp[:], in_=ctx[h][0:65, :])
                            recip = normp.tile([1, SC], f32, tag="recip")
                            nc.vector.reciprocal(recip[:], tmp[64:65, :])
                            rbc = normp.tile([64, SC], f32, tag="rbc")
                            nc.gpsimd.partition_broadcast(rbc[:], recip[:])
                            res = normp.tile([64, SC], f32, tag="res")
                            nc.vector.tensor_mul(res[:], tmp[0:64, :], rbc[:])
                            nc.sync.dma_start(
                                out[b, h * 64:(h + 1) * 64, m * SC:(m + 1) * SC],
                                res[:],
                            )

                def p2_full(b, m):
                    p2_start(b, m)
                    p2_groups(b, m, len(EXP_GROUPS))

                # q/k cols 0:512 first, then only the BATCH-0 caches — batch-1
                # cache DMAs queue after chunk 5 so they never delay batch-0
                emit_p1_q(0)
                nc.sync.dma_start(ktc_sb[:, 0], ktc[:, 0])
                nc.sync.dma_start(wk_sb[:], wk.rearrange("(ko p) m -> p ko m", p=P))
                nc.sync.dma_start(bk_sb[:], bk)
                emit_p1_q(1)
                nc.sync.dma_start(wv_sb[:], wv.rearrange("(ko p) m -> p ko m", p=P))
                nc.sync.dma_start(bv_sb[:], bv)
                _p1_proj(0, 1, wk_sb, bk_sb, kTn_w)
                _p1_proj(1, 1, wk_sb, bk_sb, kTn_w)
                emit_p1_v(0)
                emit_p1_v(1)
                # chunks 2-4 are threaded piecewise (q | k | v+transpose)
                # through the (0,0) sweep's early groups: each ~1us piece fits
                # the exp-slot wait bubble after a group, so the PE digests
                # batch-0's remaining projections without starving ACT, and
                # every kTn column is ready before the group that needs it
                p2_start(0, 0)
                p2_groups(0, 0, 1)
                emit_p1_q(2)
                # v cache + ones queue AFTER chunk 2's hsT so the kTn columns
                # gating this sweep's mid groups land sooner; the first v
                # consumer, ctx(g0), is only emitted during group 2
                nc.sync.dma_start(
                    v_sb[:, 0, 0], vc[0].rearrange("(jo p) c -> p jo c", p=P)
                )
                nc.sync.dma_start(ones_sb[:], onesp)
                nc.vector.tensor_copy(
                    out=v_sb[:, :, 1, :, 64:65],
                    in_=ones_sb[:, :, None, None].to_broadcast((P, B, VJ, 1)),
                )
                nc.vector.tensor_copy(
                    out=v_sb[:, :, 1, :, 129:130],
                    in_=ones_sb[:, :, None, None].to_broadcast((P, B, VJ, 1)),
                )
                p2_groups(0, 0, 1)
                _p1_proj(2, 1, wk_sb, bk_sb, kTn_w)
                p2_groups(0, 0, 1)
                emit_p1_v(2)
                p2_groups(0, 0, 1)
                emit_p1_q(3)
                p2_groups(0, 0, 1)
                _p1_proj(3, 1, wk_sb, bk_sb, kTn_w)
                p2_groups(0, 0, 1)
                emit_p1_v(3)
                p2_groups(0, 0, 1)
                emit_p1_q(4)
                p2_groups(0, 0, 1)
                _p1_proj(4, 1, wk_sb, bk_sb, kTn_w)
                p2_groups(0, 0, 1)
                emit_p1_v(4)
                p2_groups(0, 0, 2)
                # batch-1 projections spread inside the remaining batch-0
                # sweeps, one q/k or v piece per group batch so at most one
                # score slot is borrowed at a time and ACT stays fed
                p2_start(0, 1)
                p2_groups(0, 1, 3)
                emit_p1_q(5)
                p2_groups(0, 1, 3)
                _p1_proj(5, 1, wk_sb, bk_sb, kTn_w)
                p2_groups(0, 1, 3)
                emit_p1_v(5)
                p2_groups(0, 1, 2)
                nc.sync.dma_start(ktc_sb[:, 1], ktc[:, 1])
                nc.sync.dma_start(
                    v_sb[:, 1, 0], vc[1].rearrange("(jo p) c -> p jo c", p=P)
                )
                p2_start(0, 2)
                p2_groups(0, 2, 3)
                emit_p1_q(6)
                p2_groups(0, 2, 3)
                _p1_proj(6, 1, wk_sb, bk_sb, kTn_w)
                p2_groups(0, 2, 3)
                emit_p1_v(6)
                p2_groups(0, 2, 2)
                p2_start(0, 3)
                p2_groups(0, 3, 3)
                emit_p1_q(7)
                p2_groups(0, 3, 3)
                _p1_proj(7, 1, wk_sb, bk_sb, kTn_w)
                p2_groups(0, 3, 3)
                emit_p1_v(7)
                p2_groups(0, 3, 2)
                # (1,0) needs only chunk 5 + batch-1 caches for g0-5; chunk 8
                # (kTn cols 3584:4096, needed from g9) is emitted mid-sweep
                p2_start(1, 0)
                p2_groups(1, 0, 6)
                emit_p1_q(8)
                p2_groups(1, 0, 1)
                _p1_proj(8, 1, wk_sb, bk_sb, kTn_w)
                p2_groups(1, 0, 2)
                emit_p1_v(8)
                p2_groups(1, 0, 2)
                for m in range(1, NM):
                    p2_full(1, m)

    nc.compile()
    return nc


def get_program():
    if "nc" not in _prog_cache:
        _prog_cache["nc"] = _build_program()
    return _prog_cache["nc"]


def make_in_maps(hidden_states, kvs, Wq, bq, Wk, bk, Wv, bv, kv_weight):
    hs = np.asarray(hidden_states, np.float32).reshape(B * S, HID)
    hsT = np.ascontiguousarray(hs.T)
    kvw = np.float32(kv_weight)
    Wq = np.asarray(Wq, np.float32)
    Wk = np.asarray(Wk, np.float32)
    Wv = np.asarray(Wv, np.float32)
    bq = np.asarray(bq, np.float32)
    bk = np.asarray(bk, np.float32)
    bv = np.asarray(bv, np.float32)
    kvs = np.asarray(kvs, np.float32)
    scale = np.float32(HD ** -0.5)

    in_maps = []
    for c in range(NCORES):
        rows = slice(c * P, (c + 1) * P)
        h0, h1 = 2 * c, 2 * c + 1
        wq_c = np.ascontiguousarray((Wq[rows] * scale).T)       # [HID, 128]
        wk_c = np.ascontiguousarray(Wk[rows].T)
        wv_c = np.ascontiguousarray(Wv[rows].T)
        bq_c = np.ascontiguousarray((bq[rows] * scale).reshape(P, 1))
        bk_c = np.ascontiguousarray(bk[rows].reshape(P, 1))
        bv_c = np.ascontiguousarray(bv[rows].reshape(P, 1))
        # k cache transposed: [128 (h,d), B, SKV]
        kc = kvs[0][:, [h0, h1]] * kvw                           # [B, 2, SKV, HD]
        ktc_c = np.ascontiguousarray(kc.transpose(1, 3, 0, 2).reshape(P, B, SKV))
        # v cache with ones columns: [B, SKV, 130]
        vcache = kvs[1][:, [h0, h1]] * kvw                       # [B, 2, SKV, HD]
        vc_c = np.empty((B, SKV, 130), np.float32)
        vc_c[:, :, 0:64] = vcache[:, 0]
        vc_c[:, :, 64] = 1.0
        vc_c[:, :, 65:129] = vcache[:, 1]
        vc_c[:, :, 129] = 1.0
        in_maps.append({
            "hsT": hsT, "wq": wq_c, "wk": wk_c, "wv": wv_c,
            "bq": bq_c, "bk": bk_c, "bv": bv_c,
            "onesp": np.ones((P, 1), np.float32),
            "ktc": ktc_c, "vc": vc_c,
        })
    return in_maps


def assemble_output(results):
    full = np.empty((B, S, HID), np.float32)
    for c in range(NCORES):
        o = results[c]["out"]                                    # [B, 128, S]
        full[:, :, c * P:(c + 1) * P] = o.transpose(0, 2, 1)
    return full


def kernel(hidden_states, kvs, Wq, bq, Wk, bk, Wv, bv, kv_weight, _trace=False):
    from concourse.bass_utils import run_bass_kernel_spmd

    nc = get_program()
    in_maps = make_in_maps(hidden_states, kvs, Wq, bq, Wk, bk, Wv, bv, kv_weight)
    res = run_bass_kernel_spmd(nc, in_maps, list(range(NCORES)), trace=_trace)
    outp = assemble_output(res.results)
    if _trace:
        kernel.last_results = res
    return outp
